# revision 1
# baseline (speedup 1.0000x reference)
"""Multi-head attention (B=4, S=2048, D=1024, H=16) on 8 TRN2 NeuronCores.

Sharding: no collectives. Core c handles batch b = c//2, query-half qh = c%2
(1024 query rows). K/V projections for the batch are computed on both cores of
the pair (25% duplicated projection FLOPs, zero communication).

Math (per core), all in a "transposed" feature-major layout so softmax sums
land on free-dim columns and every operand feeds the PE without transposes:
  QT[n, q]  = (WqT tiles).T @ xT        (+ b_q per-partition via ACT bias)
  KT[n, k]  = (WkT tiles).T @ xT        (b_k provably cancels in softmax)
  V [k, n]  = (xT tiles).T @ WvT        (+ b_v via rank-1 ones matmul)
  sT[k, q]  = KT_h.T @ QT_h             (contraction d_k=64)
  eT        = exp(sT / 8)               (ACT, no max-subtraction: |s/8| < ~2.5)
  sum[q]    = ones.T @ eT               (M=1 matmul, col-packed per head pair)
  cT[d, q]  = V_h.T @ eT                (col-packed pair -> psum partitions 0-63/64-127)
  cT_norm   = cT * broadcast(1/sum)     (gpsimd partition_broadcast + DVE mul)
  out[q, n] = (cT tiles).T @ WoT + b_o  (rank-1 ones matmul for bias)

Inputs are rounded to bf16 on the host (weights/x pre-transposed); accumulation
is fp32 in PSUM. The per-core xT has its own query-half swapped to columns
0..1023 so all 8 cores run one SPMD graph (a consistent permutation of the
key/value sequence axis is a softmax no-op).
"""

import numpy as np
import ml_dtypes

BF16 = ml_dtypes.bfloat16

D = 1024      # d_model
S = 2048      # sequence length
QL = 1024     # query rows per core (half a batch)
H = 16        # heads
DK = 64       # head dim
NT = D // 128   # 8  d_model tiles
ST = S // 128   # 16 sequence tiles
QB = QL // 512  # 2  query blocks of 512

_NC_CACHE = {}


def _build_nc():
    if "nc" in _NC_CACHE:
        return _NC_CACHE["nc"]

    import concourse.bass as bass
    import concourse.mybir as mybir
    import concourse.tile as tile
    from concourse import bacc

    f32 = mybir.dt.float32
    bf16 = mybir.dt.bfloat16
    AFT = mybir.ActivationFunctionType

    # Bacc (not raw Bass): its compile() pass splits multi-wait instructions
    # into event semaphores (walrus allows one sync wait per instruction),
    # inserts gpsimd library loads, and lowers custom ISA instructions.
    nc = bacc.Bacc(name="mha8")

    xt_d = nc.dram_tensor("xt", [D, S], bf16, kind="ExternalInput")
    wqt_d = nc.dram_tensor("wqt", [D, D], bf16, kind="ExternalInput")
    wkt_d = nc.dram_tensor("wkt", [D, D], bf16, kind="ExternalInput")
    wvt_d = nc.dram_tensor("wvt", [D, D], bf16, kind="ExternalInput")
    wot_d = nc.dram_tensor("wot", [D, D], bf16, kind="ExternalInput")
    bq_d = nc.dram_tensor("bq", [128, NT], f32, kind="ExternalInput")
    bvt_d = nc.dram_tensor("bvt", [1, D], bf16, kind="ExternalInput")
    bot_d = nc.dram_tensor("bot", [1, D], bf16, kind="ExternalInput")
    out_d = nc.dram_tensor("out", [QL, D], f32, kind="ExternalOutput")

    with tile.TileContext(nc) as tc:
        with (
            tc.tile_pool(name="persist", bufs=1) as persist,
            tc.tile_pool(name="small", bufs=2) as small,
            tc.tile_pool(name="misc512", bufs=4) as misc512,
        ):
            # ---- persistent SBUF ----
            qt_sb = persist.tile([128, NT, QL], bf16)    # QT: feature-major Q
            kt_sb = persist.tile([128, NT, S], bf16)     # KT: feature-major K
            vp_sb = persist.tile([128, ST, D], bf16)     # V natural [k, n]
            ctx_sb = persist.tile([128, NT, QL], bf16)   # normalized context.T
            bq_sb = persist.tile([128, NT], f32)
            bvt_sb = persist.tile([1, D], bf16)
            bot_sb = persist.tile([1, D], bf16)
            ones_sb = persist.tile([128, 1], bf16)   # lhsT for sum matmuls (K=128, M=1)
            nc.vector.memset(ones_sb, 1.0)
            ones_row = persist.tile([1, 128], bf16)  # lhsT for rank-1 bias matmuls
            nc.vector.memset(ones_row, 1.0)

            nc.sync.dma_start(out=bq_sb, in_=bq_d[:, :])
            nc.sync.dma_start(out=bvt_sb, in_=bvt_d[:, :])
            nc.sync.dma_start(out=bot_sb, in_=bot_d[:, :])

            # ================= phase 1: projections =================
            with (
                tc.tile_pool(name="ph1w", bufs=1) as ph1w,
                tc.tile_pool(name="ps1", bufs=4, space="PSUM") as ps1,
            ):
                xt_sb = ph1w.tile([128, NT, S], bf16)
                wqt_sb = ph1w.tile([128, NT, D], bf16)
                wkt_sb = ph1w.tile([128, NT, D], bf16)
                wvt_sb = ph1w.tile([128, NT, D], bf16)

                nc.sync.dma_start(out=xt_sb, in_=xt_d[:, :].rearrange("(t p) s -> p t s", p=128))
                nc.sync.dma_start(out=wqt_sb, in_=wqt_d[:, :].rearrange("(t p) n -> p t n", p=128))
                nc.sync.dma_start(out=wkt_sb, in_=wkt_d[:, :].rearrange("(t p) n -> p t n", p=128))
                nc.sync.dma_start(out=wvt_sb, in_=wvt_d[:, :].rearrange("(t p) n -> p t n", p=128))

                # QT[n, q]: lhsT = WqT d-tile slice, rhs = xT (query half = cols 0..QL)
                for i in range(NT):
                    for jq in range(QB):
                        ps = ps1.tile([128, 512], f32, tag="ps")
                        for k in range(NT):
                            nc.tensor.matmul(
                                ps,
                                wqt_sb[:, k, i * 128:(i + 1) * 128],
                                xt_sb[:, k, jq * 512:(jq + 1) * 512],
                                start=(k == 0),
                                stop=(k == NT - 1),
                            )
                        nc.scalar.activation(
                            out=qt_sb[:, i, jq * 512:(jq + 1) * 512],
                            in_=ps,
                            func=AFT.Identity,
                            bias=bq_sb[:, i:i + 1],
                            scale=1.0,
                        )

                # KT[n, k_seq]: full sequence, no bias (b_k cancels in softmax)
                for i in range(NT):
                    for jk in range(S // 512):
                        ps = ps1.tile([128, 512], f32, tag="ps")
                        for k in range(NT):
                            nc.tensor.matmul(
                                ps,
                                wkt_sb[:, k, i * 128:(i + 1) * 128],
                                xt_sb[:, k, jk * 512:(jk + 1) * 512],
                                start=(k == 0),
                                stop=(k == NT - 1),
                            )
                        nc.vector.tensor_copy(
                            out=kt_sb[:, i, jk * 512:(jk + 1) * 512], in_=ps
                        )

                # V natural [k_seq, n]: lhsT = xT seq-slice, rhs = WvT; + ones x b_v
                for m in range(ST):
                    for jn in range(D // 512):
                        ps = ps1.tile([128, 512], f32, tag="ps")
                        for k in range(NT):
                            nc.tensor.matmul(
                                ps,
                                xt_sb[:, k, m * 128:(m + 1) * 128],
                                wvt_sb[:, k, jn * 512:(jn + 1) * 512],
                                start=(k == 0),
                                stop=False,
                            )
                        nc.tensor.matmul(
                            ps,
                            ones_row,
                            bvt_sb[:, jn * 512:(jn + 1) * 512],
                            start=False,
                            stop=True,
                        )
                        nc.vector.tensor_copy(
                            out=vp_sb[:, m, jn * 512:(jn + 1) * 512], in_=ps
                        )

            # ===== pool spanning phases 2+3: W_o tiles (DMA hidden under phase 2) =====
            from contextlib import ExitStack
            late_ctx = ExitStack()
            late = late_ctx.enter_context(tc.tile_pool(name="late", bufs=1))
            wot_sb = late.tile([128, NT, D], bf16)
            nc.sync.dma_start(out=wot_sb, in_=wot_d[:, :].rearrange("(t p) n -> p t n", p=128))

            # ================= phase 2: attention =================
            with (
                tc.tile_pool(name="expp", bufs=2) as expp,
                tc.tile_pool(name="ps_sc", bufs=2, space="PSUM") as ps_sc,
                tc.tile_pool(name="ps_ctx", bufs=2, space="PSUM") as ps_ctx,
                tc.tile_pool(name="ps_sum", bufs=2, space="PSUM") as ps_sum,
                tc.tile_pool(name="dramp", bufs=4, space="DRAM") as dramp,
            ):
                for j in range(H // 2):  # head pair (2j, 2j+1)
                    et = [None, None]
                    for hh in range(2):
                        h = 2 * j + hh
                        pb = 64 * hh  # partition base of head's features in tile j
                        e_t = expp.tile([128, ST, QL], bf16, tag="e_t")
                        et[hh] = e_t
                        for kt in range(ST):
                            ps_s = ps_sc.tile([128, QL], f32, tag="ps_s")
                            for jq in range(QB):
                                nc.tensor.matmul(
                                    ps_s[:, jq * 512:(jq + 1) * 512],
                                    kt_sb[pb:pb + 64, j, kt * 128:(kt + 1) * 128],
                                    qt_sb[pb:pb + 64, j, jq * 512:(jq + 1) * 512],
                                    start=True,
                                    stop=True,
                                )
                            nc.scalar.activation(
                                out=e_t[:, kt, :],
                                in_=ps_s,
                                func=AFT.Exp,
                                scale=0.125,
                            )

                    for jq in range(QB):
                        qs = slice(jq * 512, (jq + 1) * 512)
                        ps_c = ps_ctx.tile([128, 512], f32, tag="ps_c")
                        ps_m = ps_sum.tile([128, 512], f32, tag="ps_m")
                        for hh in range(2):
                            h = 2 * j + hh
                            pb = 64 * hh
                            for kt in range(ST):
                                # context.T: head hh -> psum partitions pb..pb+64
                                nc.tensor.matmul(
                                    ps_c[pb:pb + 64, :],
                                    vp_sb[:, kt, h * 64:(h + 1) * 64],
                                    et[hh][:, kt, qs],
                                    start=(kt == 0),
                                    stop=(kt == ST - 1),
                                    tile_position=(0, pb),
                                )
                                # softmax denominator -> psum partition pb
                                nc.tensor.matmul(
                                    ps_m[pb:pb + 1, :],
                                    ones_sb,
                                    et[hh][:, kt, qs],
                                    start=(kt == 0),
                                    stop=(kt == ST - 1),
                                    tile_position=(0, pb),
                                )

                        recip = small.tile([128, 512], f32, tag="recip")
                        rb = misc512.tile([128, 512], f32, tag="rb")
                        for hh in range(2):
                            h = 2 * j + hh
                            pb = 64 * hh
                            nc.vector.reciprocal(
                                out=recip[pb:pb + 1, :], in_=ps_m[pb:pb + 1, :]
                            )
                            rd = dramp.tile([1, 512], f32, tag="rd")
                            nc.sync.dma_start(out=rd, in_=recip[pb:pb + 1, :])
                            src_b = bass.AP(
                                tensor=rd.tensor,
                                offset=rd.offset,
                                ap=[[0, 64]] + [list(a) for a in rd.ap[1:]],
                            )
                            nc.sync.dma_start(out=rb[pb:pb + 64, :], in_=src_b)
                        nc.vector.tensor_mul(ctx_sb[:, j, qs], ps_c, rb)

            # ================= phase 3: output projection =================
            with tc.tile_pool(name="ps3", bufs=4, space="PSUM") as ps3:
                for qt in range(QL // 128):
                    for jn in range(D // 512):
                        ps = ps3.tile([128, 512], f32, tag="ps")
                        for k in range(NT):
                            nc.tensor.matmul(
                                ps,
                                ctx_sb[:, k, qt * 128:(qt + 1) * 128],
                                wot_sb[:, k, jn * 512:(jn + 1) * 512],
                                start=(k == 0),
                                stop=False,
                            )
                        nc.tensor.matmul(
                            ps,
                            ones_row,
                            bot_sb[:, jn * 512:(jn + 1) * 512],
                            start=False,
                            stop=True,
                        )
                        o_sb = misc512.tile([128, 512], f32, tag="o_sb")
                        nc.vector.tensor_copy(out=o_sb, in_=ps)
                        nc.sync.dma_start(
                            out=out_d[qt * 128:(qt + 1) * 128, jn * 512:(jn + 1) * 512],
                            in_=o_sb,
                        )
            late_ctx.close()

    nc.finalize()
    _NC_CACHE["nc"] = nc
    return nc


def _prep_in_maps(x, W_q, b_q, W_k, W_v, b_v, W_o, b_o):
    wqt = np.ascontiguousarray(W_q.T).astype(BF16)
    wkt = np.ascontiguousarray(W_k.T).astype(BF16)
    wvt = np.ascontiguousarray(W_v.T).astype(BF16)
    wot = np.ascontiguousarray(W_o.T).astype(BF16)
    bq = np.ascontiguousarray(b_q.reshape(NT, 128).T).astype(np.float32)
    bvt = b_v.reshape(1, D).astype(BF16)
    bot = b_o.reshape(1, D).astype(BF16)

    in_maps = []
    for c in range(8):
        b, qh = divmod(c, 2)
        xT = x[b].T  # [D, S]
        if qh == 0:
            xt = xT
        else:
            xt = np.concatenate([xT[:, QL:], xT[:, :QL]], axis=1)
        xt = np.ascontiguousarray(xt).astype(BF16)
        in_maps.append(
            {
                "xt": xt,
                "wqt": wqt, "wkt": wkt, "wvt": wvt, "wot": wot,
                "bq": bq, "bvt": bvt, "bot": bot,
            }
        )
    return in_maps


def _run(inputs, trace=False, trace_kwargs=None):
    from concourse import bass_utils

    nc = _build_nc()
    in_maps = _prep_in_maps(
        inputs["x"], inputs["W_q"], inputs["b_q"], inputs["W_k"],
        inputs["W_v"], inputs["b_v"], inputs["W_o"], inputs["b_o"],
    )
    kwargs = {}
    if trace:
        kwargs["trace"] = True
        if trace_kwargs:
            kwargs.update(trace_kwargs)
    res = bass_utils.run_bass_kernel_spmd(
        nc, in_maps, core_ids=list(range(8)), **kwargs
    )
    out = np.empty((4, S, D), np.float32)
    for c, r in enumerate(res.results):
        b, qh = divmod(c, 2)
        out[b, qh * QL:(qh + 1) * QL, :] = r["out"]
    return out, res


def kernel(**inputs):
    out, _ = _run(inputs, trace=False)
    return out



# revision 2
# speedup vs baseline: 1.2118x; 1.2118x over previous
"""Multi-head attention (B=4, S=2048, D=1024, H=16) on 8 TRN2 NeuronCores.

Sharding: no collectives. Core c handles batch b = c//2, query-half qh = c%2
(1024 query rows). K/V projections for the batch are computed on both cores of
the pair (25% duplicated projection FLOPs, zero communication).

v2 rewrite (from the 850us baseline, which was ~90% tensor-engine busy but
heavily HAM-throttled and spent 22% of matmul rows on softmax-denominator
matmuls):
  * The denominator now comes for free from the context matmul: W_v is
    augmented host-side with one zero column per head whose bias is 1.0, so
    V has a ones column per head and the M=65 context matmul accumulates
    sum(exp) on psum partition 64. Matmul cost is N rows (out free size)
    regardless of M, so this removes all 512 M=1 sum matmuls per core.
  * Single flat emission schedule: QT, KT prologue then a software pipeline
    of per-(head, quarter) score/exp/context units with V-projection blocks
    woven in, so the PE stream never idles (keeps the HAM clock-gate warm).
  * Normalization is deferred: context is copied psum->sbuf unnormalized
    (freeing the psum bank immediately), 1/sum is broadcast across
    partitions via a DRAM round-trip DMA, and the multiply happens in-place
    in SBUF a few pipeline steps later, off the critical path.
  * Odd heads' context lands on psum/SBUF partitions 0..63 but belongs at
    64..127 for the feature-major output projection; DVE is lane-locked, so
    a small SBUF->SBUF DMA does the partition shift.

Math (per core), feature-major so softmax sums land on free-dim columns:
  QT[n, q]  = (WqT tiles).T @ xT        (+ b_q per-partition via DVE add)
  KT[n, k]  = (WkT tiles).T @ xT        (b_k provably cancels in softmax)
  Vaug[k,m] = (xT tiles).T @ WvT_aug    (+ [b_v | 1.0] via rank-1 ones matmul)
  sT[k, q]  = KT_h.T @ QT_h             (contraction d_k=64)
  eT        = exp(sT / 8)               (ACT, no max-subtraction: |s/8| small)
  cT[d,q],Z = Vaug_h.T @ eT             (M=65: row 64 is the softmax sum Z)
  ctx       = cT * broadcast(1/Z)       (deferred, in-place in SBUF)
  out[q, n] = (ctx tiles).T @ WoT + b_o (rank-1 ones matmul for bias)

Inputs are rounded to bf16 on the host (weights/x pre-transposed); accumulation
is fp32 in PSUM. The per-core xT has its own query-half swapped to columns
0..1023 so all 8 cores run one SPMD graph (a consistent permutation of the
key/value sequence axis is a softmax no-op).
"""

import numpy as np
import ml_dtypes

BF16 = ml_dtypes.bfloat16

D = 1024      # d_model
S = 2048      # sequence length
QL = 1024     # query rows per core (half a batch)
H = 16        # heads
DK = 64       # head dim
NT = D // 128   # 8  d_model tiles
ST = S // 128   # 16 sequence tiles
DA = H * 65     # 1040 augmented V feature columns (64 + ones col per head)
VB = 260        # V projection block width (4 heads x 65)
NU = H * 4      # 64 pipeline units: (head, quarter of the key sequence)

_NC_CACHE = {}


def _build_nc():
    if "nc" in _NC_CACHE:
        return _NC_CACHE["nc"]

    import concourse.bass as bass
    import concourse.mybir as mybir
    import concourse.tile as tile
    from concourse import bacc

    f32 = mybir.dt.float32
    bf16 = mybir.dt.bfloat16
    AFT = mybir.ActivationFunctionType

    nc = bacc.Bacc(name="mha8v2")

    xt_d = nc.dram_tensor("xt", [D, S], bf16, kind="ExternalInput")
    wqt_d = nc.dram_tensor("wqt", [D, D], bf16, kind="ExternalInput")
    wkt_d = nc.dram_tensor("wkt", [D, D], bf16, kind="ExternalInput")
    wvt_d = nc.dram_tensor("wvt", [D, DA], bf16, kind="ExternalInput")
    wot_d = nc.dram_tensor("wot", [D, D], bf16, kind="ExternalInput")
    bq_d = nc.dram_tensor("bq", [128, NT], f32, kind="ExternalInput")
    bvt_d = nc.dram_tensor("bvt", [1, DA], bf16, kind="ExternalInput")
    bot_d = nc.dram_tensor("bot", [1, D], bf16, kind="ExternalInput")
    out_d = nc.dram_tensor("out", [QL, D], f32, kind="ExternalOutput")

    with tile.TileContext(nc) as tc:
        with (
            tc.tile_pool(name="persist", bufs=1) as persist,
            tc.tile_pool(name="xpool", bufs=1) as xpool,
            tc.tile_pool(name="wpool", bufs=2) as wpool,
            tc.tile_pool(name="epool", bufs=2) as epool,
            tc.tile_pool(name="npool", bufs=2) as npool,
            tc.tile_pool(name="opool", bufs=2) as opool,
            tc.tile_pool(name="dramp", bufs=4, space="DRAM") as dramp,
            tc.tile_pool(name="pp", bufs=2, space="PSUM") as pp,
            tc.tile_pool(name="pss", bufs=2, space="PSUM") as pss,
            tc.tile_pool(name="psc", bufs=2, space="PSUM") as psc,
        ):
            # ---- persistent SBUF ----
            qt_sb = persist.tile([128, NT, QL], bf16)    # QT: feature-major Q
            kt_sb = persist.tile([128, NT, S], bf16)     # KT: feature-major K
            vp_sb = persist.tile([128, ST, DA], bf16)    # V augmented [k, 16*(64+1)]
            ctx_sb = persist.tile([128, NT, QL], bf16)   # context.T (normalized in place)
            bq_sb = persist.tile([128, NT], f32)
            bvt_sb = persist.tile([1, DA], bf16)
            bot_sb = persist.tile([1, D], bf16)
            ones_row = persist.tile([1, 128], bf16)  # lhsT for rank-1 bias matmuls
            nc.vector.memset(ones_row, 1.0)

            # ---- initial DMAs, in consumption order ----
            nc.sync.dma_start(out=bq_sb, in_=bq_d[:, :])
            xt_sb = xpool.tile([128, NT, S], bf16)
            # query half first: QT needs cols 0..QL only
            nc.sync.dma_start(
                out=xt_sb[:, :, 0:QL],
                in_=xt_d[:, 0:QL].rearrange("(t p) s -> p t s", p=128),
            )
            wqt_sb = wpool.tile([128, NT, D], bf16, tag="w")
            for i in range(NT):  # chunked so QT(i=0) can start early
                nc.sync.dma_start(
                    out=wqt_sb[:, :, i * 128:(i + 1) * 128],
                    in_=wqt_d[:, i * 128:(i + 1) * 128].rearrange(
                        "(t p) n -> p t n", p=128
                    ),
                )
            wkt_sb = wpool.tile([128, NT, D], bf16, tag="w")
            nc.sync.dma_start(
                out=wkt_sb, in_=wkt_d[:, :].rearrange("(t p) n -> p t n", p=128)
            )
            nc.sync.dma_start(
                out=xt_sb[:, :, QL:S],
                in_=xt_d[:, QL:S].rearrange("(t p) s -> p t s", p=128),
            )
            nc.sync.dma_start(out=bvt_sb, in_=bvt_d[:, :])
            nc.sync.dma_start(out=bot_sb, in_=bot_d[:, :])

            # ================= prologue: Q and K projections =================
            # QT[n, q]: lhsT = WqT d-tile slice, rhs = xT (query half)
            for i in range(NT):
                for jq in range(2):
                    ps = pp.tile([128, 512], f32, tag="p")
                    for k in range(NT):
                        nc.tensor.matmul(
                            ps,
                            wqt_sb[:, k, i * 128:(i + 1) * 128],
                            xt_sb[:, k, jq * 512:(jq + 1) * 512],
                            start=(k == 0),
                            stop=(k == NT - 1),
                        )
                    # bias add on DVE (keeps ACT free for exp, no table thrash)
                    nc.vector.tensor_scalar_add(
                        qt_sb[:, i, jq * 512:(jq + 1) * 512], ps, bq_sb[:, i:i + 1]
                    )

            # wvt rotates into wqt's slot (waits for QT's last read of wqt)
            wvt_sb = wpool.tile([128, NT, DA], bf16, tag="w")
            nc.sync.dma_start(
                out=wvt_sb, in_=wvt_d[:, :].rearrange("(t p) n -> p t n", p=128)
            )

            # KT[n, k_seq]: full sequence, no bias (b_k cancels in softmax)
            for i in range(NT):
                for jk in range(S // 512):
                    ps = pp.tile([128, 512], f32, tag="p")
                    for k in range(NT):
                        nc.tensor.matmul(
                            ps,
                            wkt_sb[:, k, i * 128:(i + 1) * 128],
                            xt_sb[:, k, jk * 512:(jk + 1) * 512],
                            start=(k == 0),
                            stop=(k == NT - 1),
                        )
                    nc.vector.tensor_copy(
                        out=kt_sb[:, i, jk * 512:(jk + 1) * 512], in_=ps
                    )

            # wot rotates into wkt's slot (waits for KT's last read of wkt)
            wot_sb = wpool.tile([128, NT, D], bf16, tag="w")
            nc.sync.dma_start(
                out=wot_sb, in_=wot_d[:, :].rearrange("(t p) n -> p t n", p=128)
            )

            # ================= unit pipeline =================
            # unit u = (head h = u//4, quarter q = u%4): kt tiles 4q..4q+3.
            # S(u): scores + exp into e_t(u).  C(u): context accumulation.
            # S leads C by 2 units; V-projection blocks are emitted on demand.
            v_done = set()        # (m, jn) V blocks already emitted
            e_tiles = {}          # u -> e_t tile
            c_tiles = {}          # h -> (ps_c_jq0, ps_c_jq1)
            pending_muls = []     # (emit_at_iter, fn)

            def emit_v_block(m, jn):
                if (m, jn) in v_done:
                    return
                v_done.add((m, jn))
                ps = pp.tile([128, 512], f32, tag="p")
                for k in range(NT):
                    nc.tensor.matmul(
                        ps[:, 0:VB],
                        xt_sb[:, k, m * 128:(m + 1) * 128],
                        wvt_sb[:, k, jn * VB:(jn + 1) * VB],
                        start=(k == 0),
                        stop=False,
                    )
                nc.tensor.matmul(
                    ps[:, 0:VB],
                    ones_row,
                    bvt_sb[:, jn * VB:(jn + 1) * VB],
                    start=False,
                    stop=True,
                )
                nc.vector.tensor_copy(
                    out=vp_sb[:, m, jn * VB:(jn + 1) * VB], in_=ps[:, 0:VB]
                )

            def emit_scores(u):
                h, q = divmod(u, 4)
                j, pb = h // 2, 64 * (h % 2)
                e_t = epool.tile([128, 4, QL], bf16, tag="e", name=f"e_{u}")
                e_tiles[u] = e_t
                for ktl in range(4):
                    kt = 4 * q + ktl
                    ps_s = pss.tile([128, QL], f32, tag="s", name=f"ss_{u}_{ktl}")
                    for jq in range(2):
                        nc.tensor.matmul(
                            ps_s[:, jq * 512:(jq + 1) * 512],
                            kt_sb[pb:pb + 64, j, kt * 128:(kt + 1) * 128],
                            qt_sb[pb:pb + 64, j, jq * 512:(jq + 1) * 512],
                            start=True,
                            stop=True,
                        )
                    nc.scalar.activation(
                        out=e_t[:, ktl, :], in_=ps_s, func=AFT.Exp, scale=0.125
                    )

            def emit_ctx(u, it):
                h, q = divmod(u, 4)
                for m in range(4 * q, 4 * q + 4):
                    emit_v_block(m, h // 4)
                if q == 0:
                    c_tiles[h] = (
                        psc.tile([128, 512], f32, tag="c", name=f"c_{h}_0"),
                        psc.tile([128, 512], f32, tag="c", name=f"c_{h}_1"),
                    )
                e_t = e_tiles.pop(u)
                for jq in range(2):
                    ps_c = c_tiles[h][jq]
                    for ktl in range(4):
                        kt = 4 * q + ktl
                        nc.tensor.matmul(
                            ps_c[0:65, :],
                            vp_sb[:, kt, h * 65:(h + 1) * 65],
                            e_t[:, ktl, jq * 512:(jq + 1) * 512],
                            start=(kt == 0),
                            stop=(kt == ST - 1),
                        )
                if q == 3:
                    emit_norm(h, it)

            def emit_norm(h, it):
                """Copy unnormalized ctx out of psum, broadcast 1/Z across
                partitions via DRAM, multiply in place later (deferred)."""
                j, pb = h // 2, 64 * (h % 2)
                ps0, ps1 = c_tiles.pop(h)
                for jq, ps_c in ((0, ps0), (1, ps1)):
                    qs = slice(jq * 512, (jq + 1) * 512)
                    if pb == 0:
                        nc.vector.tensor_copy(
                            out=ctx_sb[0:64, j, qs], in_=ps_c[0:64, :]
                        )
                    else:
                        tmp = npool.tile([64, 512], bf16, tag="t", name=f"t_{h}_{jq}")
                        nc.vector.tensor_copy(out=tmp, in_=ps_c[0:64, :])
                        # DVE is lane-locked; DMA does the partition shift
                        nc.sync.dma_start(out=ctx_sb[64:128, j, qs], in_=tmp)
                    recip = npool.tile([128, 512], f32, tag="r", name=f"r_{h}_{jq}")
                    nc.vector.reciprocal(
                        out=recip[64:65, :], in_=ps_c[64:65, :]
                    )
                    rd = dramp.tile([1, 512], f32, tag="rd", name=f"rd_{h}_{jq}")
                    nc.sync.dma_start(out=rd, in_=recip[64:65, :])
                    rb = npool.tile([128, 512], f32, tag="b", name=f"b_{h}_{jq}")
                    src_b = bass.AP(
                        tensor=rd.tensor,
                        offset=rd.offset,
                        ap=[[0, 64]] + [list(a) for a in rd.ap[1:]],
                    )
                    nc.sync.dma_start(out=rb[pb:pb + 64, :], in_=src_b)
                    pending_muls.append(
                        (
                            it + 4,
                            lambda j=j, pb=pb, qs=qs, rb=rb: nc.vector.tensor_mul(
                                ctx_sb[pb:pb + 64, j, qs],
                                ctx_sb[pb:pb + 64, j, qs],
                                rb[pb:pb + 64, :],
                            ),
                        )
                    )

            for it in range(NU + 2):
                while pending_muls and pending_muls[0][0] <= it:
                    pending_muls.pop(0)[1]()
                if it >= 2:
                    emit_ctx(it - 2, it)
                if it < NU:
                    emit_scores(it)
            while pending_muls:
                pending_muls.pop(0)[1]()

            # ================= output projection =================
            for qt in range(QL // 128):
                for jn in range(D // 512):
                    ps = pp.tile([128, 512], f32, tag="p")
                    for k in range(NT):
                        nc.tensor.matmul(
                            ps,
                            ctx_sb[:, k, qt * 128:(qt + 1) * 128],
                            wot_sb[:, k, jn * 512:(jn + 1) * 512],
                            start=(k == 0),
                            stop=False,
                        )
                    nc.tensor.matmul(
                        ps,
                        ones_row,
                        bot_sb[:, jn * 512:(jn + 1) * 512],
                        start=False,
                        stop=True,
                    )
                    o_sb = opool.tile([128, 512], f32, tag="o")
                    nc.vector.tensor_copy(out=o_sb, in_=ps)
                    nc.sync.dma_start(
                        out=out_d[qt * 128:(qt + 1) * 128, jn * 512:(jn + 1) * 512],
                        in_=o_sb,
                    )

    nc.finalize()
    _NC_CACHE["nc"] = nc
    return nc


def _prep_in_maps(x, W_q, b_q, W_k, W_v, b_v, W_o, b_o):
    wqt = np.ascontiguousarray(W_q.T).astype(BF16)
    wkt = np.ascontiguousarray(W_k.T).astype(BF16)
    wot = np.ascontiguousarray(W_o.T).astype(BF16)
    # augmented W_v.T: per head 64 data columns + 1 zero column whose bias is
    # 1.0, so V gets a ones column and the context matmul also computes the
    # softmax denominator on psum partition 64
    wvt = np.zeros((D, DA), dtype=BF16)
    bvt = np.zeros((1, DA), dtype=np.float32)
    wv_t = np.asarray(W_v.T, dtype=np.float32)
    for h in range(H):
        wvt[:, h * 65:h * 65 + 64] = wv_t[:, h * 64:(h + 1) * 64].astype(BF16)
        bvt[0, h * 65:h * 65 + 64] = b_v[h * 64:(h + 1) * 64]
        bvt[0, h * 65 + 64] = 1.0
    bvt = bvt.astype(BF16)
    bq = np.ascontiguousarray(b_q.reshape(NT, 128).T).astype(np.float32)
    bot = b_o.reshape(1, D).astype(BF16)

    in_maps = []
    for c in range(8):
        b, qh = divmod(c, 2)
        xT = x[b].T  # [D, S]
        if qh == 0:
            xt = xT
        else:
            xt = np.concatenate([xT[:, QL:], xT[:, :QL]], axis=1)
        xt = np.ascontiguousarray(xt).astype(BF16)
        in_maps.append(
            {
                "xt": xt,
                "wqt": wqt, "wkt": wkt, "wvt": wvt, "wot": wot,
                "bq": bq, "bvt": bvt, "bot": bot,
            }
        )
    return in_maps


def _run(inputs, trace=False, trace_kwargs=None):
    from concourse import bass_utils

    nc = _build_nc()
    in_maps = _prep_in_maps(
        inputs["x"], inputs["W_q"], inputs["b_q"], inputs["W_k"],
        inputs["W_v"], inputs["b_v"], inputs["W_o"], inputs["b_o"],
    )
    kwargs = {}
    if trace:
        kwargs["trace"] = True
        if trace_kwargs:
            kwargs.update(trace_kwargs)
    res = bass_utils.run_bass_kernel_spmd(
        nc, in_maps, core_ids=list(range(8)), **kwargs
    )
    out = np.empty((4, S, D), np.float32)
    for c, r in enumerate(res.results):
        b, qh = divmod(c, 2)
        out[b, qh * QL:(qh + 1) * QL, :] = r["out"]
    return out, res


def kernel(**inputs):
    out, _ = _run(inputs, trace=False)
    return out


# revision 6
# speedup vs baseline: 1.3627x; 1.1245x over previous
"""Multi-head attention (B=4, S=2048, D=1024, H=16) on 8 TRN2 NeuronCores.

Sharding: no collectives. Core c handles batch b = c//2, query-half qh = c%2
(1024 query rows). K/V projections for the batch are computed on both cores of
the pair (25% duplicated projection FLOPs, zero communication).

v3 (from 850us baseline -> 701us v2 -> this):
  * Softmax denominator comes free from the context matmul: W_v is augmented
    host-side with one zero column per head whose bias is 1.0, so V carries a
    ones column and the M=65 context matmul accumulates sum(exp) on psum
    partition 64. Matmul cost is N rows (out free size) regardless of M, so
    this removes all M=1 denominator matmuls.
  * 1/Z is broadcast across partitions with a rank-1 PE matmul
    (ones.T @ recip_row -> psum), not a DRAM round-trip; the normalize
    multiply then runs in place in SBUF one pipeline step later.
  * Flat software-pipelined emission: QT/KT prologue, then per-(head,
    quarter) units S(u)=scores+exp, C(u)=context, with V-projection blocks
    prefetched ~1 per iteration as PE filler so the tensor engine never
    idles long enough to trip the HAM clock gate.
  * Odd heads' context needs psum partitions 64..127 but M=65 matmuls can
    only write partition base 0; DVE is lane-locked, so a small SBUF->SBUF
    DMA shifts the 64-row block up.

Math (per core), feature-major so softmax sums land on free-dim columns:
  QT[n, q]  = (WqT tiles).T @ xT        (+ b_q per-partition via DVE add)
  KT[n, k]  = (WkT tiles).T @ xT        (b_k provably cancels in softmax)
  Vaug[k,m] = (xT tiles).T @ WvT_aug    (+ [b_v | 1.0] via rank-1 ones matmul)
  sT[k, q]  = KT_h.T @ QT_h             (contraction d_k=64)
  eT        = exp(sT / 8)               (ACT, no max-subtraction: |s/8| small)
  cT[d,q],Z = Vaug_h.T @ eT             (M=65: row 64 is the softmax sum Z)
  ctx       = cT * rank1_broadcast(1/Z) (deferred, in-place in SBUF)
  out[q, n] = (ctx tiles).T @ WoT + b_o (rank-1 ones matmul for bias)

Inputs are rounded to bf16 on the host (weights/x pre-transposed); accumulation
is fp32 in PSUM. The per-core xT has its own query-half swapped to columns
0..1023 so all 8 cores run one SPMD graph (a consistent permutation of the
key/value sequence axis is a softmax no-op).
"""

import numpy as np
import ml_dtypes

BF16 = ml_dtypes.bfloat16

D = 1024      # d_model
S = 2048      # sequence length
QL = 1024     # query rows per core (half a batch)
H = 16        # heads
DK = 64       # head dim
NT = D // 128   # 8  d_model tiles
ST = S // 128   # 16 sequence tiles
DA = H * 65     # 1040 augmented V feature columns (64 + ones col per head)
VB = 260        # V projection block width (4 heads x 65)
NU = H * 4      # 64 pipeline units: (head, quarter of the key sequence)

_NC_CACHE = {}


def _build_nc():
    if "nc" in _NC_CACHE:
        return _NC_CACHE["nc"]

    import concourse.bass as bass
    import concourse.mybir as mybir
    import concourse.tile as tile
    from concourse import bacc

    f32 = mybir.dt.float32
    bf16 = mybir.dt.bfloat16
    AFT = mybir.ActivationFunctionType

    nc = bacc.Bacc(name="mha8v3")

    xt_d = nc.dram_tensor("xt", [D, S], bf16, kind="ExternalInput")
    wqt_d = nc.dram_tensor("wqt", [D, D], bf16, kind="ExternalInput")
    wkt_d = nc.dram_tensor("wkt", [D, D], bf16, kind="ExternalInput")
    wvt_d = nc.dram_tensor("wvt", [D, DA], bf16, kind="ExternalInput")
    wot_d = nc.dram_tensor("wot", [D, D], bf16, kind="ExternalInput")
    bq_d = nc.dram_tensor("bq", [128, NT], f32, kind="ExternalInput")
    bvt_d = nc.dram_tensor("bvt", [1, DA], bf16, kind="ExternalInput")
    bot_d = nc.dram_tensor("bot", [1, D], bf16, kind="ExternalInput")
    out_d = nc.dram_tensor("out", [QL, D], f32, kind="ExternalOutput")

    with tile.TileContext(nc) as tc:
        with (
            tc.tile_pool(name="persist", bufs=1) as persist,
            tc.tile_pool(name="xpool", bufs=1) as xpool,
            tc.tile_pool(name="wpool", bufs=2) as wpool,
            tc.tile_pool(name="epool", bufs=3) as epool,
            tc.tile_pool(name="npool", bufs=2) as npool,
            tc.tile_pool(name="opool", bufs=2) as opool,
            tc.tile_pool(name="pp", bufs=2, space="PSUM") as pp,
            tc.tile_pool(name="pss", bufs=2, space="PSUM") as pss,
            tc.tile_pool(name="psc", bufs=2, space="PSUM") as psc,
        ):
            # ---- persistent SBUF ----
            qt_sb = persist.tile([128, NT, QL], bf16)    # QT: feature-major Q
            kt_sb = persist.tile([128, NT, S], bf16)     # KT: feature-major K
            vp_sb = persist.tile([128, ST, DA], bf16)    # V augmented [k, 16*(64+1)]
            ctx_sb = persist.tile([128, NT, QL], bf16)   # context.T (normalized in place)
            bq_sb = persist.tile([128, NT], f32)
            bvt_sb = persist.tile([1, DA], bf16)
            bot_sb = persist.tile([1, D], bf16)
            ones_row = persist.tile([1, 128], bf16)  # lhsT for rank-1 bias matmuls
            nc.vector.memset(ones_row, 1.0)
            ones65 = persist.tile([65, 128], bf16)   # row 64: lhsT for 1/Z bcast
            nc.vector.memset(ones65, 1.0)

            # ---- initial DMAs, in consumption order ----
            nc.sync.dma_start(out=bq_sb, in_=bq_d[:, :])
            wqt_sb = wpool.tile([128, NT, D], bf16, tag="w")
            nc.sync.dma_start(
                out=wqt_sb[:, :, 0:128],
                in_=wqt_d[:, 0:128].rearrange("(t p) n -> p t n", p=128),
            )
            xt_sb = xpool.tile([128, NT, S], bf16)
            # query half first: QT needs cols 0..QL only
            nc.sync.dma_start(
                out=xt_sb[:, :, 0:QL],
                in_=xt_d[:, 0:QL].rearrange("(t p) s -> p t s", p=128),
            )
            for i in range(1, NT):  # chunked so QT(i) can start as chunks land
                nc.sync.dma_start(
                    out=wqt_sb[:, :, i * 128:(i + 1) * 128],
                    in_=wqt_d[:, i * 128:(i + 1) * 128].rearrange(
                        "(t p) n -> p t n", p=128
                    ),
                )
            wkt_sb = wpool.tile([128, NT, D], bf16, tag="w")
            nc.sync.dma_start(
                out=wkt_sb, in_=wkt_d[:, :].rearrange("(t p) n -> p t n", p=128)
            )
            nc.sync.dma_start(
                out=xt_sb[:, :, QL:S],
                in_=xt_d[:, QL:S].rearrange("(t p) s -> p t s", p=128),
            )
            nc.sync.dma_start(out=bvt_sb, in_=bvt_d[:, :])
            nc.sync.dma_start(out=bot_sb, in_=bot_d[:, :])

            # ================= prologue: Q and K projections =================
            # QT[n, q]: lhsT = WqT d-tile slice, rhs = xT (query half)
            for i in range(NT):
                for jq in range(2):
                    ps = pp.tile([128, 512], f32, tag="p")
                    for k in range(NT):
                        nc.tensor.matmul(
                            ps,
                            wqt_sb[:, k, i * 128:(i + 1) * 128],
                            xt_sb[:, k, jq * 512:(jq + 1) * 512],
                            start=(k == 0),
                            stop=(k == NT - 1),
                        )
                    # bias add on DVE (keeps ACT exp-only: no table thrash)
                    nc.vector.tensor_scalar_add(
                        qt_sb[:, i, jq * 512:(jq + 1) * 512], ps, bq_sb[:, i:i + 1]
                    )

            # wvt rotates into wqt's slot (waits for QT's last read of wqt)
            wvt_sb = wpool.tile([128, NT, DA], bf16, tag="w")
            nc.sync.dma_start(
                out=wvt_sb, in_=wvt_d[:, :].rearrange("(t p) n -> p t n", p=128)
            )

            # KT[n, k_seq]: full sequence, no bias (b_k cancels in softmax)
            for i in range(NT):
                for jk in range(S // 512):
                    ps = pp.tile([128, 512], f32, tag="p")
                    for k in range(NT):
                        nc.tensor.matmul(
                            ps,
                            wkt_sb[:, k, i * 128:(i + 1) * 128],
                            xt_sb[:, k, jk * 512:(jk + 1) * 512],
                            start=(k == 0),
                            stop=(k == NT - 1),
                        )
                    nc.vector.tensor_copy(
                        out=kt_sb[:, i, jk * 512:(jk + 1) * 512], in_=ps
                    )

            # wot rotates into wkt's slot (waits for KT's last read of wkt)
            wot_sb = wpool.tile([128, NT, D], bf16, tag="w")
            nc.sync.dma_start(
                out=wot_sb, in_=wot_d[:, :].rearrange("(t p) n -> p t n", p=128)
            )

            # ================= unit pipeline =================
            # unit u = (head h = u//4, quarter q = u%4): kt tiles 4q..4q+3.
            # S(u): scores + exp into e_t(u).  C(u): context accumulation.
            # S leads C by 2 units; V blocks are prefetched as PE filler.
            v_done = set()        # (m, jn) V blocks already emitted
            e_tiles = {}          # u -> e_t tile
            c_tiles = {}          # h -> (ps_c_jq0, ps_c_jq1)
            pending_norm = []     # (flush_at_iter, fn) rank-1 bcast + in-place mul

            def emit_v_block(m, jn):
                if (m, jn) in v_done:
                    return False
                v_done.add((m, jn))
                ps = pp.tile([128, 512], f32, tag="p", name=f"v_{m}_{jn}")
                for k in range(NT):
                    nc.tensor.matmul(
                        ps[:, 0:VB],
                        xt_sb[:, k, m * 128:(m + 1) * 128],
                        wvt_sb[:, k, jn * VB:(jn + 1) * VB],
                        start=(k == 0),
                        stop=False,
                    )
                nc.tensor.matmul(
                    ps[:, 0:VB],
                    ones_row,
                    bvt_sb[:, jn * VB:(jn + 1) * VB],
                    start=False,
                    stop=True,
                )
                nc.vector.tensor_copy(
                    out=vp_sb[:, m, jn * VB:(jn + 1) * VB], in_=ps[:, 0:VB]
                )
                return True

            # prefetch order: all (m, jn) by first-consumption time
            v_queue = [(m, jn) for jn in range(4) for m in range(ST)]

            def emit_scores_chunk(u, ktl):
                h, q = divmod(u, 4)
                j, pb = h // 2, 64 * (h % 2)
                kt = 4 * q + ktl
                e_t = e_tiles[u]
                ps_s = pss.tile([128, QL], f32, tag="s", name=f"ss_{u}_{ktl}")
                for jq in range(2):
                    nc.tensor.matmul(
                        ps_s[:, jq * 512:(jq + 1) * 512],
                        kt_sb[pb:pb + 64, j, kt * 128:(kt + 1) * 128],
                        qt_sb[pb:pb + 64, j, jq * 512:(jq + 1) * 512],
                        start=True,
                        stop=True,
                    )
                nc.scalar.activation(
                    out=e_t[:, ktl, :], in_=ps_s, func=AFT.Exp, scale=0.125
                )

            def emit_ctx_chunk(u, x):
                """x in 0..3 -> (jq, ktl pair)"""
                h, q = divmod(u, 4)
                jq, kp = divmod(x, 2)
                ps_c = c_tiles[h][jq]
                e_t = e_tiles[u]
                for ktl in (2 * kp, 2 * kp + 1):
                    kt = 4 * q + ktl
                    nc.tensor.matmul(
                        ps_c[0:65, :],
                        vp_sb[:, kt, h * 65:(h + 1) * 65],
                        e_t[:, ktl, jq * 512:(jq + 1) * 512],
                        start=(kt == 0),
                        stop=(kt == ST - 1),
                    )

            def emit_norm(h, it):
                """Copy unnormalized ctx out of psum + 1/Z recip now; the
                rank-1 broadcast and in-place multiply are deferred one
                iteration so the PE never waits on the DVE recip."""
                j, pb = h // 2, 64 * (h % 2)
                ps0, ps1 = c_tiles.pop(h)
                for jq, ps_c in ((0, ps0), (1, ps1)):
                    qs = slice(jq * 512, (jq + 1) * 512)
                    if pb == 0:
                        nc.vector.tensor_copy(
                            out=ctx_sb[0:64, j, qs], in_=ps_c[0:64, :]
                        )
                    else:
                        tmp = npool.tile([64, 512], bf16, tag="t", name=f"t_{h}_{jq}")
                        nc.vector.tensor_copy(out=tmp, in_=ps_c[0:64, :])
                        # DVE is lane-locked; DMA does the partition shift
                        nc.sync.dma_start(out=ctx_sb[64:128, j, qs], in_=tmp)
                    # bf16: must match ones65 dtype for the rank-1 matmul
                    recip = npool.tile([128, 512], bf16, tag="r", name=f"r_{h}_{jq}")
                    with nc.allow_low_precision(
                        reason="1/Z broadcast via bf16 rank-1 matmul; 0.2% "
                        "scale noise per (head, q), well inside the gate"
                    ):
                        nc.vector.reciprocal(
                            out=recip[64:65, :], in_=ps_c[64:65, :]
                        )

                    def do_norm(j=j, pb=pb, qs=qs, recip=recip, h=h, jq=jq):
                        ps_b = pp.tile([128, 512], f32, tag="p", name=f"pb_{h}_{jq}")
                        nc.tensor.matmul(
                            ps_b[pb:pb + 64, :],
                            ones65[64:65, 0:64],
                            recip[64:65, :],
                            start=True,
                            stop=True,
                            tile_position=(64, pb),
                        )
                        nc.vector.tensor_mul(
                            ctx_sb[pb:pb + 64, j, qs],
                            ctx_sb[pb:pb + 64, j, qs],
                            ps_b[pb:pb + 64, :],
                        )

                    pending_norm.append((it + 1, do_norm))

            for it in range(NU + 2):
                cu, su = it - 2, it
                # deferred rank-1 broadcasts + in-place normalize muls
                while pending_norm and pending_norm[0][0] <= it:
                    pending_norm.pop(0)[1]()
                if 0 <= cu:
                    h, q = divmod(cu, 4)
                    # V blocks this C-unit consumes (no-ops if prefetched)
                    for m in range(4 * q, 4 * q + 4):
                        emit_v_block(m, h // 4)
                    if q == 0:
                        c_tiles[h] = (
                            psc.tile([128, 512], f32, tag="c", name=f"c_{h}_0"),
                            psc.tile([128, 512], f32, tag="c", name=f"c_{h}_1"),
                        )
                    for x in range(4):
                        emit_ctx_chunk(cu, x)
                    if q == 3:
                        e_tiles.pop(cu - 3), e_tiles.pop(cu - 2), e_tiles.pop(cu - 1)
                        e_tiles.pop(cu)
                        emit_norm(h, it)
                if su < NU:
                    e_tiles[su] = epool.tile(
                        [128, 4, QL], bf16, tag="e", name=f"e_{su}"
                    )
                    emit_scores_chunk(su, 0)
                    emit_scores_chunk(su, 1)
                    # ~1 prefetched V block as PE filler between score chunks
                    while v_queue:
                        blk = v_queue.pop(0)
                        if emit_v_block(*blk):
                            break
                    emit_scores_chunk(su, 2)
                    emit_scores_chunk(su, 3)
            while pending_norm:
                pending_norm.pop(0)[1]()

            # ================= output projection =================
            for qt in range(QL // 128):
                for jn in range(D // 512):
                    ps = pp.tile([128, 512], f32, tag="p")
                    for k in range(NT):
                        nc.tensor.matmul(
                            ps,
                            ctx_sb[:, k, qt * 128:(qt + 1) * 128],
                            wot_sb[:, k, jn * 512:(jn + 1) * 512],
                            start=(k == 0),
                            stop=False,
                        )
                    nc.tensor.matmul(
                        ps,
                        ones_row,
                        bot_sb[:, jn * 512:(jn + 1) * 512],
                        start=False,
                        stop=True,
                    )
                    o_sb = opool.tile([128, 512], f32, tag="o")
                    nc.vector.tensor_copy(out=o_sb, in_=ps)
                    nc.sync.dma_start(
                        out=out_d[qt * 128:(qt + 1) * 128, jn * 512:(jn + 1) * 512],
                        in_=o_sb,
                    )

    nc.finalize()
    _NC_CACHE["nc"] = nc
    return nc


def _prep_in_maps(x, W_q, b_q, W_k, W_v, b_v, W_o, b_o):
    wqt = np.ascontiguousarray(W_q.T).astype(BF16)
    wkt = np.ascontiguousarray(W_k.T).astype(BF16)
    wot = np.ascontiguousarray(W_o.T).astype(BF16)
    # augmented W_v.T: per head 64 data columns + 1 zero column whose bias is
    # 1.0, so V gets a ones column and the context matmul also computes the
    # softmax denominator on psum partition 64
    wvt = np.zeros((D, DA), dtype=BF16)
    bvt = np.zeros((1, DA), dtype=np.float32)
    wv_t = np.asarray(W_v.T, dtype=np.float32)
    for h in range(H):
        wvt[:, h * 65:h * 65 + 64] = wv_t[:, h * 64:(h + 1) * 64].astype(BF16)
        bvt[0, h * 65:h * 65 + 64] = b_v[h * 64:(h + 1) * 64]
        bvt[0, h * 65 + 64] = 1.0
    bvt = bvt.astype(BF16)
    bq = np.ascontiguousarray(b_q.reshape(NT, 128).T).astype(np.float32)
    bot = b_o.reshape(1, D).astype(BF16)

    in_maps = []
    for c in range(8):
        b, qh = divmod(c, 2)
        xT = x[b].T  # [D, S]
        if qh == 0:
            xt = xT
        else:
            xt = np.concatenate([xT[:, QL:], xT[:, :QL]], axis=1)
        xt = np.ascontiguousarray(xt).astype(BF16)
        in_maps.append(
            {
                "xt": xt,
                "wqt": wqt, "wkt": wkt, "wvt": wvt, "wot": wot,
                "bq": bq, "bvt": bvt, "bot": bot,
            }
        )
    return in_maps


def _run(inputs, trace=False, trace_kwargs=None):
    from concourse import bass_utils

    nc = _build_nc()
    in_maps = _prep_in_maps(
        inputs["x"], inputs["W_q"], inputs["b_q"], inputs["W_k"],
        inputs["W_v"], inputs["b_v"], inputs["W_o"], inputs["b_o"],
    )
    kwargs = {}
    if trace:
        kwargs["trace"] = True
        if trace_kwargs:
            kwargs.update(trace_kwargs)
    res = bass_utils.run_bass_kernel_spmd(
        nc, in_maps, core_ids=list(range(8)), **kwargs
    )
    out = np.empty((4, S, D), np.float32)
    for c, r in enumerate(res.results):
        b, qh = divmod(c, 2)
        out[b, qh * QL:(qh + 1) * QL, :] = r["out"]
    return out, res


def kernel(**inputs):
    out, _ = _run(inputs, trace=False)
    return out
